# revision 44
# baseline (speedup 1.0000x reference)
"""Trainium2 Bass kernel for DiscreteGCNLayer.

Computation (per batch b):
    dw      = ternary_quantize(weight, s=0.01)            # [256, 256]
    support = x[b] @ dw                                   # [2048, 256]
    out[b]  = relu(adj[b] @ support + bias)               # [2048, 256]

Strategy: data-parallel over the batch dim (8 batches -> 8 NeuronCores),
weight/bias replicated.  Inputs are staged host-side in bf16 with layouts
chosen so the device kernel needs no on-chip transposes and every DMA
moves >=4KB-contiguous lines:

  xt_d   [256, 2048]            x[b]^T        (stage-1 lhsT tiles)
  adjt_d [4, 128, 16, 512]      adj[b]^T tiled as [nb, p, mc, j] with
                                adjt[nb, p, mc, j] = adj[b][nb*512+j, mc*128+p]
  out_d  [256, 2048]            out[b]^T, un-transposed on the host

Per core the kernel computes out^T so no operand ever needs an on-chip
transpose:
  stage 1: psum[m, o] += xt-tile[128i, 128m]^T @ dw[128i, 256o]
  stage 2: psum[128o-half, 512n] += lhsT=support[128m, 128o-half] (natural
           SBUF layout) @ rhs=adjT quarter [128m, 512n] streamed from HBM
  bias+relu ride the PSUM->SBUF eviction: in the out^T orientation bias[o]
  is a per-partition scalar, so ACT's activation(Relu, bias=...) (and
  DVE's tensor_scalar add+max for the sibling half) fuse it for free.

Schedule highlights (tuned against the TimelineSim cost model):
  - weight DMA first on the SP queue; dw quantize per-chunk on DVE so
    stage 1 unblocks at ~4.5us; junk matmuls cover the PE p-state ramp.
  - stage 2's first n-block is interleaved with stage 1 quarter-by-quarter
    so the PE never drains between stages; nb+1's adjT quarters prefetch
    while nb's are consumed (6-deep pool).
  - the last n-block accumulates oh1 as two half-width psum groups in
    separate banks; the three groups close staggered so the final
    relu->store->sem chain is short.  PE is gapless from ~4.6us to the
    last matmul.

All matmuls are bf16 (1 cycle/row on the PE, rel-err ~3.4e-3 vs the fp32
reference, comfortably inside the 2e-2 gate; the ternary threshold
compare stays fp32).  HBM traffic per core drops from 20.25MB (fp32) to
~10.3MB, and the PE does 160 real matmuls instead of 614 (no transposes,
no bias matmuls).
"""

import sys

import numpy as np

if "/opt/trn_rl_repo" not in sys.path:
    sys.path.insert(0, "/opt/trn_rl_repo")

B = 8
N = 2048
DIN = 256
DOUT = 256
P = 128
NBW = 512          # stage-2 moving-dim window (n columns per psum group)
NNB = N // NBW     # 4 n-blocks
MB = N // P        # 16 contraction chunks (stage 2)
IB = DIN // P      # 2 contraction chunks (stage 1)
QW = 4             # m-chunks per adjT quarter DMA
NQ = MB // QW      # 4 quarter DMAs per n-block
XW = 4             # xt column windows (stage-1 early start)
OH = DOUT // P     # 2 output-partition halves
SPARSITY = 0.01
WARMUP = 6         # junk matmuls covering PE ramp + input-DMA latency

_NC = None


def _build_nc():
    from contextlib import ExitStack

    import concourse.bass as bass
    import concourse.mybir as mybir
    import concourse.tile as tile
    from concourse import bacc

    F32 = mybir.dt.float32
    BF16 = mybir.dt.bfloat16
    Alu = mybir.AluOpType

    nc = bacc.Bacc()
    xt_d = nc.dram_tensor("xt", [DIN, N], BF16, kind="ExternalInput")
    adjt_d = nc.dram_tensor("adjt", [NNB, P, MB, NBW], BF16, kind="ExternalInput")
    w_d = nc.dram_tensor("weight", [DIN, DOUT], F32, kind="ExternalInput")
    b_d = nc.dram_tensor("bias", [DOUT], F32, kind="ExternalInput")
    out_d = nc.dram_tensor("out", [DOUT, N], BF16, kind="ExternalOutput")

    with tile.TileContext(nc) as tc, ExitStack() as ctx:
        singles = ctx.enter_context(tc.tile_pool(name="singles", bufs=1))
        aq_pool = ctx.enter_context(tc.tile_pool(name="aq", bufs=6))
        ot_pool = ctx.enter_context(tc.tile_pool(name="ot", bufs=8))
        psum_s1 = ctx.enter_context(tc.tile_pool(name="ps1", bufs=4, space="PSUM"))
        psum_s2 = ctx.enter_context(tc.tile_pool(name="ps2", bufs=4, space="PSUM"))

        # ---- inputs in flight ------------------------------------------
        # weight rides the FRONT of the SP queue as two per-chunk DMAs:
        # within-queue order is deterministic, so chunk 0 lands first and
        # its quantize chain (which gates every stage-1 matmul) starts one
        # transfer earlier.
        w_sb = singles.tile([P, IB, DOUT], F32)
        nc.sync.dma_start(out=w_sb, in_=w_d[:].rearrange("(c p) o -> p c o", p=P))

        # xt in XW column windows so stage 1 starts on window 0.  The
        # windows are interleaved with nb0's adjT quarters on the SP queue
        # (see below) so stage 2's first block can start right behind
        # stage 1's first chunks.
        xt_sb = singles.tile([P, IB, N], BF16)
        xt_r = xt_d[:].rearrange("(c p) m -> p c m", p=P)
        WN = N // XW

        def start_xt(w):
            nc.sync.dma_start(
                out=xt_sb[:, :, w * WN : (w + 1) * WN],
                in_=xt_r[:, :, w * WN : (w + 1) * WN],
            )

        # ---- PE warm-up ------------------------------------------------
        # The cost model's p-state ramp needs ~3us of continuous PE
        # activity to reach full clock; the real work is gated on the
        # xt/weight DMAs anyway, so spend the wait ramping.  The junk
        # memset goes on DVE (idle at t=0; the quantize chain can't start
        # until the weight DMA lands anyway) so the first junk matmul
        # issues as early as possible.
        junk = singles.tile([P, NBW], BF16)
        nc.vector.memset(junk, 1.0)
        jrelu = singles.tile([P, 8], BF16)
        nc.scalar.activation(jrelu, junk[:, 0:8], mybir.ActivationFunctionType.Relu)
        for i in range(WARMUP):
            jp = psum_s2.tile([P, NBW], F32, tag="ps2")
            nc.tensor.matmul(jp, lhsT=junk[:, 0:P], rhs=junk, start=True, stop=True)

        # ---- ternary-quantized weight: dw = ((w > s) - (w < -s)) * s ---
        # Fully per-chunk: chunk 0's three DVE ops complete before chunk 1's
        # start, so stage 1's first matmul is unblocked as early as possible.
        dw = singles.tile([P, IB, DOUT], BF16)
        tpos = singles.tile([P, IB, DOUT], F32)
        tneg = singles.tile([P, IB, DOUT], F32)
        for ic in range(IB):
            nc.vector.tensor_scalar(
                out=tpos[:, ic, :], in0=w_sb[:, ic, :],
                scalar1=SPARSITY, scalar2=SPARSITY,
                op0=Alu.is_gt, op1=Alu.mult,
            )
            nc.vector.tensor_scalar(
                out=tneg[:, ic, :], in0=w_sb[:, ic, :],
                scalar1=-SPARSITY, scalar2=SPARSITY,
                op0=Alu.is_lt, op1=Alu.mult,
            )
            nc.vector.tensor_sub(dw[:, ic, :], tpos[:, ic, :], tneg[:, ic, :])

        # ---- stage-2 adjT prefetch stream ------------------------------
        aq_tiles = {}

        def start_aq(nb, q):
            aq = aq_pool.tile([P, QW, NBW], BF16, tag="aq", name=f"aq{nb}_{q}")
            nc.sync.dma_start(out=aq, in_=adjt_d[nb, :, q * QW : (q + 1) * QW, :])
            aq_tiles[(nb, q)] = aq

        # SP-queue DMA order tuned to consumption order: stage-1 quartet w
        # needs xt window w just before stage 2 (nb0, q=w-1) needs its
        # quarter, so xt windows run one slot ahead of nb0's quarters.
        start_xt(0)
        start_xt(1)
        start_aq(0, 0)
        start_xt(2)
        start_aq(0, 1)
        start_xt(3)
        start_aq(0, 2)
        start_aq(0, 3)
        bias_sb = singles.tile([P, OH], F32)
        nc.sync.dma_start(out=bias_sb, in_=b_d[:].rearrange("(c p) -> p c", p=P))

        # ---- fused stage 1 + stage 2 -----------------------------------
        # stage 1: support[m-chunk][p, o] = sum_i x[., i] dw[i, o]
        # stage 2: outT[oh, nb*512+n] = relu(sum_m support[m, oh*] adjT + b)
        support = singles.tile([P, MB, DOUT], BF16)

        def s1_chunk(mc):
            sp = psum_s1.tile([P, DOUT], F32, tag="ps1")
            for ic in range(IB):
                nc.tensor.matmul(
                    sp,
                    lhsT=xt_sb[:, ic, mc * P : (mc + 1) * P],
                    rhs=dw[:, ic, :],
                    start=(ic == 0),
                    stop=(ic == IB - 1),
                )
            if mc % 2 == 0:
                nc.vector.tensor_copy(support[:, mc, :], sp)
            else:
                nc.scalar.copy(support[:, mc, :], sp)

        s2_psums = {}

        def s2_quarter(nb, q):
            """Matmuls for stage-2 block nb over m-chunks q*QW..q*QW+3."""
            if q == 0:
                s2_psums[nb] = [
                    psum_s2.tile([P, NBW], F32, tag="ps2", name=f"po{nb}_{oh}")
                    for oh in range(OH)
                ]
            po = s2_psums[nb]
            aq = aq_tiles[(nb, q)]
            for k in range(QW):
                mc = q * QW + k
                rhs = aq[:, k, :]
                for oh in range(OH):
                    nc.tensor.matmul(
                        po[oh],
                        lhsT=support[:, mc, oh * P : (oh + 1) * P],
                        rhs=rhs,
                        start=(mc == 0),
                        stop=(mc == MB - 1),
                    )

        def s2_close(nb):
            """bias+relu evictions in parallel (oh0 on ACT, oh1 on DVE),
            stores on the ACT queue (the only other HWDGE queue is SP,
            which is owned by the adjT prefetch stream)."""
            po = s2_psums[nb]
            ot0 = ot_pool.tile([P, NBW], BF16, tag="ot")
            nc.scalar.activation(
                ot0, po[0], mybir.ActivationFunctionType.Relu,
                bias=bias_sb[:, 0:1],
            )
            ot1 = ot_pool.tile([P, NBW], BF16, tag="ot")
            nc.vector.tensor_scalar(
                out=ot1, in0=po[1], scalar1=bias_sb[:, 1:2], scalar2=0.0,
                op0=Alu.add, op1=Alu.max,
            )
            nc.scalar.dma_start(
                out=out_d[0:P, nb * NBW : (nb + 1) * NBW], in_=ot0
            )
            nc.scalar.dma_start(
                out=out_d[P : 2 * P, nb * NBW : (nb + 1) * NBW], in_=ot1
            )

        # nb0 rides along with stage 1, quarter by quarter; nb1's quarters
        # are prefetched as nb0's are consumed.  The first quartet is
        # emitted ic0-major (all four ic0 matmuls into four open psum
        # groups, then the ic1 matmuls): dw chunk 1 lands one DVE op after
        # chunk 0, and this ordering keeps the PE fed across that window.
        for k in range(QW):
            s1_chunk(k)
        s2_quarter(0, 0)
        start_aq(1, 0)
        for q in range(1, NQ):
            for k in range(QW):
                s1_chunk(q * QW + k)
            s2_quarter(0, q)
            start_aq(1, q)
        s2_close(0)

        # middle blocks: steady-state stream, one block of lookahead.
        for nb in range(1, NNB - 1):
            for q in range(NQ):
                start_aq(nb + 1, q)
                s2_quarter(nb, q)
            s2_close(nb)

        # Last block: oh0 accumulates full-width in the ps2 ring; oh1
        # accumulates as two independent half-width groups in separate PSUM
        # banks (the ps1 ring is idle by now).  In the final quarter oh0's
        # matmuls are emitted first, then oh1-half-a's, then oh1-half-b's,
        # so the three groups close staggered and the very last
        # relu->store->sem chain only carries a [128, 256] piece.
        nb = NNB - 1
        H = NBW // 2
        po0 = psum_s2.tile([P, NBW], F32, tag="ps2", name="po3_0")
        po1h = [psum_s1.tile([P, H], F32, tag="ps1", name=f"po3_1{h}")
                for h in range(2)]

        def last_mms(mc, aq, k, groups):
            for g in groups:
                if g == 0:
                    nc.tensor.matmul(
                        po0, lhsT=support[:, mc, 0:P], rhs=aq[:, k, :],
                        start=(mc == 0), stop=(mc == MB - 1),
                    )
                else:
                    h = g - 1
                    nc.tensor.matmul(
                        po1h[h],
                        lhsT=support[:, mc, P : 2 * P],
                        rhs=aq[:, k, h * H : (h + 1) * H],
                        start=(mc == 0), stop=(mc == MB - 1),
                    )

        for q in range(NQ - 1):
            aq = aq_tiles[(nb, q)]
            for k in range(QW):
                last_mms(q * QW + k, aq, k, (0, 1, 2))
        q = NQ - 1
        aq = aq_tiles[(nb, q)]
        for g in (0, 1, 2):
            for k in range(QW):
                last_mms(q * QW + k, aq, k, (g,))

        # closes, in stop order: po0 (ACT relu, SP store), po1h[0] (DVE,
        # ACT-queue store), po1h[1] (ACT relu, SP store).
        ot0 = ot_pool.tile([P, NBW], BF16, tag="ot")
        nc.scalar.activation(
            ot0, po0, mybir.ActivationFunctionType.Relu, bias=bias_sb[:, 0:1]
        )
        nc.sync.dma_start(out=out_d[0:P, nb * NBW : (nb + 1) * NBW], in_=ot0)
        ot1 = ot_pool.tile([P, NBW], BF16, tag="ot")
        nc.vector.tensor_scalar(
            out=ot1[:, 0:H], in0=po1h[0], scalar1=bias_sb[:, 1:2], scalar2=0.0,
            op0=Alu.add, op1=Alu.max,
        )
        nc.scalar.activation(
            ot1[:, H:NBW], po1h[1], mybir.ActivationFunctionType.Relu,
            bias=bias_sb[:, 1:2],
        )
        nc.sync.dma_start(out=out_d[P : 2 * P, nb * NBW : (nb + 1) * NBW], in_=ot1)

    nc.compile()
    return nc


def _get_nc():
    global _NC
    if _NC is None:
        _NC = _build_nc()
    return _NC


def kernel(x, adj, weight, bias, _trace=False):
    import ml_dtypes
    from concourse import bass_utils

    bf16 = ml_dtypes.bfloat16
    x = np.asarray(x, dtype=np.float32)
    adj = np.asarray(adj, dtype=np.float32)
    weight = np.ascontiguousarray(np.asarray(weight, dtype=np.float32))
    bias = np.ascontiguousarray(np.asarray(bias, dtype=np.float32))

    nc = _get_nc()
    in_maps = []
    for b in range(B):
        xt = np.ascontiguousarray(x[b].T).astype(bf16)
        # adjt[nb, p, mc, j] = adj[b][nb*512 + j, mc*128 + p]
        adjt = np.ascontiguousarray(
            adj[b].reshape(NNB, NBW, MB, P).transpose(0, 3, 2, 1)
        ).astype(bf16)
        in_maps.append({"xt": xt, "adjt": adjt, "weight": weight, "bias": bias})

    try:
        res = bass_utils.run_bass_kernel_spmd(
            nc, in_maps, core_ids=list(range(B)), trace=_trace
        )
    except Exception:
        # one retry: a previously wedged NeuronCore surfaces as a transient
        # NRT_EXEC_UNIT_UNRECOVERABLE on the first execution after it
        res = bass_utils.run_bass_kernel_spmd(
            nc, in_maps, core_ids=list(range(B)), trace=_trace
        )
    out = np.stack(
        [np.asarray(r["out"]).astype(np.float32).T for r in res.results], axis=0
    )
    if _trace:
        return out, res
    return out


# revision 46
# speedup vs baseline: 1.0115x; 1.0115x over previous
"""Trainium2 Bass kernel for DiscreteGCNLayer.

Computation (per batch b):
    dw      = ternary_quantize(weight, s=0.01)            # [256, 256]
    support = x[b] @ dw                                   # [2048, 256]
    out[b]  = relu(adj[b] @ support + bias)               # [2048, 256]

Strategy: data-parallel over the batch dim (8 batches -> 8 NeuronCores),
weight/bias replicated.  Inputs are staged host-side in bf16 with layouts
chosen so the device kernel needs no on-chip transposes and every DMA
moves >=4KB-contiguous lines:

  xt_d   [256, 2048]            x[b]^T        (stage-1 lhsT tiles)
  adjt_d [4, 128, 16, 512]      adj[b]^T tiled as [nb, p, mc, j] with
                                adjt[nb, p, mc, j] = adj[b][nb*512+j, mc*128+p]
  out_d  [256, 2048]            out[b]^T, un-transposed on the host

Per core the kernel computes out^T so no operand ever needs an on-chip
transpose:
  stage 1: psum[m, o] += xt-tile[128i, 128m]^T @ dw[128i, 256o]
  stage 2: psum[128o-half, 512n] += lhsT=support[128m, 128o-half] (natural
           SBUF layout) @ rhs=adjT quarter [128m, 512n] streamed from HBM
  bias+relu ride the PSUM->SBUF eviction: in the out^T orientation bias[o]
  is a per-partition scalar, so ACT's activation(Relu, bias=...) (and
  DVE's tensor_scalar add+max for the sibling half) fuse it for free.

Schedule highlights (tuned against the TimelineSim cost model):
  - weight DMA first on the SP queue; dw quantize per-chunk on DVE so
    stage 1 unblocks at ~4.5us; junk matmuls cover the PE p-state ramp.
  - stage 2's first n-block is interleaved with stage 1 quarter-by-quarter
    so the PE never drains between stages; nb+1's adjT quarters prefetch
    while nb's are consumed (6-deep pool).
  - the last n-block accumulates oh1 as two half-width psum groups in
    separate banks; the three groups close staggered so the final
    relu->store->sem chain is short.  PE is gapless from ~4.6us to the
    last matmul.

All matmuls are bf16 (1 cycle/row on the PE, rel-err ~3.4e-3 vs the fp32
reference, comfortably inside the 2e-2 gate; the ternary threshold
compare stays fp32).  HBM traffic per core drops from 20.25MB (fp32) to
~10.3MB, and the PE does 160 real matmuls instead of 614 (no transposes,
no bias matmuls).
"""

import sys

import numpy as np

if "/opt/trn_rl_repo" not in sys.path:
    sys.path.insert(0, "/opt/trn_rl_repo")

B = 8
N = 2048
DIN = 256
DOUT = 256
P = 128
NBW = 512          # stage-2 moving-dim window (n columns per psum group)
NNB = N // NBW     # 4 n-blocks
MB = N // P        # 16 contraction chunks (stage 2)
IB = DIN // P      # 2 contraction chunks (stage 1)
QW = 4             # m-chunks per adjT quarter DMA
NQ = MB // QW      # 4 quarter DMAs per n-block
XW = 4             # xt column windows (stage-1 early start)
OH = DOUT // P     # 2 output-partition halves
SPARSITY = 0.01
WARMUP = 6         # junk matmuls covering PE ramp + input-DMA latency

_NC = None


def _build_nc():
    from contextlib import ExitStack

    import concourse.bass as bass
    import concourse.mybir as mybir
    import concourse.tile as tile
    from concourse import bacc

    F32 = mybir.dt.float32
    F16 = mybir.dt.float16
    BF16 = mybir.dt.bfloat16
    Alu = mybir.AluOpType

    nc = bacc.Bacc()
    xt_d = nc.dram_tensor("xt", [DIN, N], BF16, kind="ExternalInput")
    adjt_d = nc.dram_tensor("adjt", [NNB, P, MB, NBW], BF16, kind="ExternalInput")
    w_d = nc.dram_tensor("weight", [DIN, DOUT], F16, kind="ExternalInput")
    b_d = nc.dram_tensor("bias", [DOUT], F32, kind="ExternalInput")
    out_d = nc.dram_tensor("out", [DOUT, N], BF16, kind="ExternalOutput")

    with tile.TileContext(nc) as tc, ExitStack() as ctx:
        singles = ctx.enter_context(tc.tile_pool(name="singles", bufs=1))
        aq_pool = ctx.enter_context(tc.tile_pool(name="aq", bufs=6))
        ot_pool = ctx.enter_context(tc.tile_pool(name="ot", bufs=8))
        psum_s1 = ctx.enter_context(tc.tile_pool(name="ps1", bufs=4, space="PSUM"))
        psum_s2 = ctx.enter_context(tc.tile_pool(name="ps2", bufs=4, space="PSUM"))

        # ---- inputs in flight ------------------------------------------
        # weight rides the FRONT of the SP queue: within-queue order is
        # deterministic, so it lands first and its quantize chain (which
        # gates every stage-1 matmul) starts as early as possible.
        w_sb = singles.tile([P, IB, DOUT], F16)
        nc.sync.dma_start(out=w_sb, in_=w_d[:].rearrange("(c p) o -> p c o", p=P))

        # xt in XW column windows so stage 1 starts on window 0.  The
        # windows are interleaved with nb0's adjT quarters on the SP queue
        # (see below) so stage 2's first block can start right behind
        # stage 1's first chunks.
        xt_sb = singles.tile([P, IB, N], BF16)
        xt_r = xt_d[:].rearrange("(c p) m -> p c m", p=P)
        WN = N // XW

        def start_xt(w):
            nc.sync.dma_start(
                out=xt_sb[:, :, w * WN : (w + 1) * WN],
                in_=xt_r[:, :, w * WN : (w + 1) * WN],
            )

        # ---- PE warm-up ------------------------------------------------
        # The cost model's p-state ramp needs ~3us of continuous PE
        # activity to reach full clock; the real work is gated on the
        # xt/weight DMAs anyway, so spend the wait ramping.  The junk
        # memset goes on DVE (idle at t=0; the quantize chain can't start
        # until the weight DMA lands anyway) so the first junk matmul
        # issues as early as possible.
        junk = singles.tile([P, NBW], BF16)
        nc.vector.memset(junk, 1.0)
        jrelu = singles.tile([P, 8], BF16)
        nc.scalar.activation(jrelu, junk[:, 0:8], mybir.ActivationFunctionType.Relu)
        for i in range(WARMUP):
            jp = psum_s2.tile([P, NBW], F32, tag="ps2")
            nc.tensor.matmul(jp, lhsT=junk[:, 0:P], rhs=junk, start=True, stop=True)

        # ---- ternary-quantized weight: dw = ((w > s) - (w < -s)) * s ---
        # w is staged fp16: half the gating DMA, and the all-2-byte quantize
        # ops run in DVE's 2x mode.  (bf16 w would flip ~0.1% of borderline
        # ternary decisions -> 1.6e-2 rel err; fp16's flip band is ~1000x
        # narrower, measured 3.8e-3.)  Fully per-chunk: chunk 0's three DVE
        # ops complete before chunk 1's start, so stage 1's first matmul is
        # unblocked as early as possible.
        dw = singles.tile([P, IB, DOUT], BF16)
        tpos = singles.tile([P, IB, DOUT], F16)
        tneg = singles.tile([P, IB, DOUT], F16)
        for ic in range(IB):
            nc.vector.tensor_scalar(
                out=tpos[:, ic, :], in0=w_sb[:, ic, :],
                scalar1=SPARSITY, scalar2=SPARSITY,
                op0=Alu.is_gt, op1=Alu.mult,
            )
            nc.vector.tensor_scalar(
                out=tneg[:, ic, :], in0=w_sb[:, ic, :],
                scalar1=-SPARSITY, scalar2=SPARSITY,
                op0=Alu.is_lt, op1=Alu.mult,
            )
            nc.vector.tensor_sub(dw[:, ic, :], tpos[:, ic, :], tneg[:, ic, :])

        # ---- stage-2 adjT prefetch stream ------------------------------
        aq_tiles = {}

        def start_aq(nb, q):
            aq = aq_pool.tile([P, QW, NBW], BF16, tag="aq", name=f"aq{nb}_{q}")
            nc.sync.dma_start(out=aq, in_=adjt_d[nb, :, q * QW : (q + 1) * QW, :])
            aq_tiles[(nb, q)] = aq

        # SP-queue DMA order tuned to consumption order: stage-1 quartet w
        # needs xt window w just before stage 2 (nb0, q=w-1) needs its
        # quarter, so xt windows run one slot ahead of nb0's quarters.
        start_xt(0)
        start_xt(1)
        start_aq(0, 0)
        start_xt(2)
        start_aq(0, 1)
        start_xt(3)
        start_aq(0, 2)
        start_aq(0, 3)
        bias_sb = singles.tile([P, OH], F32)
        nc.sync.dma_start(out=bias_sb, in_=b_d[:].rearrange("(c p) -> p c", p=P))

        # ---- fused stage 1 + stage 2 -----------------------------------
        # stage 1: support[m-chunk][p, o] = sum_i x[., i] dw[i, o]
        # stage 2: outT[oh, nb*512+n] = relu(sum_m support[m, oh*] adjT + b)
        support = singles.tile([P, MB, DOUT], BF16)

        def s1_chunk(mc):
            sp = psum_s1.tile([P, DOUT], F32, tag="ps1")
            for ic in range(IB):
                nc.tensor.matmul(
                    sp,
                    lhsT=xt_sb[:, ic, mc * P : (mc + 1) * P],
                    rhs=dw[:, ic, :],
                    start=(ic == 0),
                    stop=(ic == IB - 1),
                )
            if mc % 2 == 0:
                nc.vector.tensor_copy(support[:, mc, :], sp)
            else:
                nc.scalar.copy(support[:, mc, :], sp)

        s2_psums = {}

        def s2_quarter(nb, q):
            """Matmuls for stage-2 block nb over m-chunks q*QW..q*QW+3."""
            if q == 0:
                s2_psums[nb] = [
                    psum_s2.tile([P, NBW], F32, tag="ps2", name=f"po{nb}_{oh}")
                    for oh in range(OH)
                ]
            po = s2_psums[nb]
            aq = aq_tiles[(nb, q)]
            for k in range(QW):
                mc = q * QW + k
                rhs = aq[:, k, :]
                for oh in range(OH):
                    nc.tensor.matmul(
                        po[oh],
                        lhsT=support[:, mc, oh * P : (oh + 1) * P],
                        rhs=rhs,
                        start=(mc == 0),
                        stop=(mc == MB - 1),
                    )

        def s2_close(nb):
            """bias+relu evictions in parallel (oh0 on ACT, oh1 on DVE),
            stores on the ACT queue (the only other HWDGE queue is SP,
            which is owned by the adjT prefetch stream)."""
            po = s2_psums[nb]
            ot0 = ot_pool.tile([P, NBW], BF16, tag="ot")
            nc.scalar.activation(
                ot0, po[0], mybir.ActivationFunctionType.Relu,
                bias=bias_sb[:, 0:1],
            )
            ot1 = ot_pool.tile([P, NBW], BF16, tag="ot")
            nc.vector.tensor_scalar(
                out=ot1, in0=po[1], scalar1=bias_sb[:, 1:2], scalar2=0.0,
                op0=Alu.add, op1=Alu.max,
            )
            nc.scalar.dma_start(
                out=out_d[0:P, nb * NBW : (nb + 1) * NBW], in_=ot0
            )
            nc.scalar.dma_start(
                out=out_d[P : 2 * P, nb * NBW : (nb + 1) * NBW], in_=ot1
            )

        # nb0 rides along with stage 1, quarter by quarter; nb1's quarters
        # are prefetched as nb0's are consumed.  The first quartet is
        # emitted ic0-major (all four ic0 matmuls into four open psum
        # groups, then the ic1 matmuls): dw chunk 1 lands one DVE op after
        # chunk 0, and this ordering keeps the PE fed across that window.
        for k in range(QW):
            s1_chunk(k)
        s2_quarter(0, 0)
        start_aq(1, 0)
        for q in range(1, NQ):
            for k in range(QW):
                s1_chunk(q * QW + k)
            s2_quarter(0, q)
            start_aq(1, q)
        s2_close(0)

        # middle blocks: steady-state stream, one block of lookahead.
        for nb in range(1, NNB - 1):
            for q in range(NQ):
                start_aq(nb + 1, q)
                s2_quarter(nb, q)
            s2_close(nb)

        # Last block: oh0 accumulates full-width in the ps2 ring; oh1
        # accumulates as two independent half-width groups in separate PSUM
        # banks (the ps1 ring is idle by now).  In the final quarter oh0's
        # matmuls are emitted first, then oh1-half-a's, then oh1-half-b's,
        # so the three groups close staggered and the very last
        # relu->store->sem chain only carries a [128, 256] piece.
        nb = NNB - 1
        H = NBW // 2
        po0 = psum_s2.tile([P, NBW], F32, tag="ps2", name="po3_0")
        po1h = [psum_s1.tile([P, H], F32, tag="ps1", name=f"po3_1{h}")
                for h in range(2)]

        def last_mms(mc, aq, k, groups):
            for g in groups:
                if g == 0:
                    nc.tensor.matmul(
                        po0, lhsT=support[:, mc, 0:P], rhs=aq[:, k, :],
                        start=(mc == 0), stop=(mc == MB - 1),
                    )
                else:
                    h = g - 1
                    nc.tensor.matmul(
                        po1h[h],
                        lhsT=support[:, mc, P : 2 * P],
                        rhs=aq[:, k, h * H : (h + 1) * H],
                        start=(mc == 0), stop=(mc == MB - 1),
                    )

        for q in range(NQ - 1):
            aq = aq_tiles[(nb, q)]
            for k in range(QW):
                last_mms(q * QW + k, aq, k, (0, 1, 2))
        q = NQ - 1
        aq = aq_tiles[(nb, q)]
        for g in (0, 1, 2):
            for k in range(QW):
                last_mms(q * QW + k, aq, k, (g,))

        # closes, in stop order: po0 (ACT relu, SP store), po1h[0] (DVE,
        # ACT-queue store), po1h[1] (ACT relu, SP store).
        ot0 = ot_pool.tile([P, NBW], BF16, tag="ot")
        nc.scalar.activation(
            ot0, po0, mybir.ActivationFunctionType.Relu, bias=bias_sb[:, 0:1]
        )
        nc.sync.dma_start(out=out_d[0:P, nb * NBW : (nb + 1) * NBW], in_=ot0)
        ot1 = ot_pool.tile([P, NBW], BF16, tag="ot")
        nc.vector.tensor_scalar(
            out=ot1[:, 0:H], in0=po1h[0], scalar1=bias_sb[:, 1:2], scalar2=0.0,
            op0=Alu.add, op1=Alu.max,
        )
        nc.scalar.activation(
            ot1[:, H:NBW], po1h[1], mybir.ActivationFunctionType.Relu,
            bias=bias_sb[:, 1:2],
        )
        nc.sync.dma_start(out=out_d[P : 2 * P, nb * NBW : (nb + 1) * NBW], in_=ot1)

    nc.compile()
    return nc


def _get_nc():
    global _NC
    if _NC is None:
        _NC = _build_nc()
    return _NC


def kernel(x, adj, weight, bias, _trace=False):
    import ml_dtypes
    from concourse import bass_utils

    bf16 = ml_dtypes.bfloat16
    x = np.asarray(x, dtype=np.float32)
    adj = np.asarray(adj, dtype=np.float32)
    weight = np.ascontiguousarray(np.asarray(weight, dtype=np.float32)).astype(np.float16)
    bias = np.ascontiguousarray(np.asarray(bias, dtype=np.float32))

    nc = _get_nc()
    in_maps = []
    for b in range(B):
        xt = np.ascontiguousarray(x[b].T).astype(bf16)
        # adjt[nb, p, mc, j] = adj[b][nb*512 + j, mc*128 + p]
        adjt = np.ascontiguousarray(
            adj[b].reshape(NNB, NBW, MB, P).transpose(0, 3, 2, 1)
        ).astype(bf16)
        in_maps.append({"xt": xt, "adjt": adjt, "weight": weight, "bias": bias})

    try:
        res = bass_utils.run_bass_kernel_spmd(
            nc, in_maps, core_ids=list(range(B)), trace=_trace
        )
    except Exception:
        # one retry: a previously wedged NeuronCore surfaces as a transient
        # NRT_EXEC_UNIT_UNRECOVERABLE on the first execution after it
        res = bass_utils.run_bass_kernel_spmd(
            nc, in_maps, core_ids=list(range(B)), trace=_trace
        )
    out = np.stack(
        [np.asarray(r["out"]).astype(np.float32).T for r in res.results], axis=0
    )
    if _trace:
        return out, res
    return out
